# revision 23
# baseline (speedup 1.0000x reference)
"""HB-LSTM cell fused Trainium2 kernel, data-parallel over 8 NeuronCores.

Computes, for gate order (f, i, o, u, k):
    pre  = x @ Wx[g].T + bx[g] + h_prev @ Uh[g].T + bh[g]
    f,i,o,u = sigmoid(pre[0..3]);  c = tanh(pre[4])
    kp = u*c + (1-u)*kp_prev
    k  = f*k_prev + i*kp
    h  = o*tanh(k)
Returns (h, k, kp), each [B, H] float32.

Sharding: batch dim B=65536 split across 8 cores (8192 rows each); weight
stacks replicated to every core.

Per-core structure (8 groups of 8 b-tiles of 128 batch rows):
  - Weight preamble: per i-chunk staged SWDGE load (cast f32->bf16) +
    one whole-tile xbar transpose per chunk yields all matmul rhs tiles;
    tanh-gate (g=4) weights/bias pre-scaled by 2 so ONE sigmoid covers
    all 1280 gate cols (tanh(x) = 2*sigmoid(2x) - 1, fixed up on DVE).
  - x/h loaded row-major into one fused tile (cast to bf16 in the SWDGE
    DMA), then ONE whole-group xbar transpose yields all 32 feature-major
    lhsT tiles of the group; transposes triple-buffered so a group's
    transpose never waits on the matmul stream two groups back.
  - Per b-tile: 5-gate pre-activations accumulate in one [128,1280] PSUM
    tile: bias via K=1 ones-matmul (start), then 12 bf16 matmuls, then
    one sigmoid on ACT into fp16 gates.
  - Elementwise tail entirely in fp16 at group granularity; k_prev /
    kp_prev cast to fp16 in the load DMA; outputs stored as fp16
    (upcast to f32 on host).
"""

import numpy as np

import concourse.bacc as bacc
import concourse.mybir as mybir
from concourse import tile
from concourse.bass_utils import run_bass_kernel_spmd

N_CORES = 8
B = 65536
IN = 256
H = 256
G5 = 5
BL = B // N_CORES          # rows per core
NT = BL // 128             # 64 b-tiles per core
GROUP = 8                  # b-tiles per group
NG = NT // GROUP
DG = G5 * H                # 1280 = all-gate column span
F32 = mybir.dt.float32
BF16 = mybir.dt.bfloat16
F16 = mybir.dt.float16
GDT = BF16                 # GEMM compute dtype
DT = F16                   # elementwise-tail dtype
AF = mybir.ActivationFunctionType
ALU = mybir.AluOpType

PSUM_BUFS = 2
TR_BUFS = 3

_CACHE = {}


def _build():
    if "nc" in _CACHE:
        return _CACHE["nc"]

    nc = bacc.Bacc("TRN2", target_bir_lowering=False, debug=False,
                   num_devices=N_CORES)

    x_d = nc.dram_tensor("x", [BL, IN], F32, kind="ExternalInput")
    h_d = nc.dram_tensor("h_prev", [BL, H], F32, kind="ExternalInput")
    k_d = nc.dram_tensor("k_prev", [BL, H], F32, kind="ExternalInput")
    kp_d = nc.dram_tensor("kp_prev", [BL, H], F32, kind="ExternalInput")
    wx_d = nc.dram_tensor("Wx", [G5, H, IN], F32, kind="ExternalInput")
    bx_d = nc.dram_tensor("bx", [G5, H], F32, kind="ExternalInput")
    uh_d = nc.dram_tensor("Uh", [G5, H, H], F32, kind="ExternalInput")
    bh_d = nc.dram_tensor("bh", [G5, H], F32, kind="ExternalInput")
    ho_d = nc.dram_tensor("h_out", [BL, H], DT, kind="ExternalOutput")
    ko_d = nc.dram_tensor("k_out", [BL, H], DT, kind="ExternalOutput")
    kpo_d = nc.dram_tensor("kp_out", [BL, H], DT, kind="ExternalOutput")

    with tile.TileContext(nc) as tc:
        with tc.tile_pool(name="const", bufs=1) as cpool:
            # WTt[c][ip, side, g, hc, p] bf16: transposed weight stacks,
            # i-chunk c on partitions. Views WT[side, c] = [128, 1280] rhs.
            WTt = [cpool.tile([128, 2, G5, 2, 128], GDT, name=f"WT{c}",
                              tag=f"WT{c}") for c in range(2)]
            WT = {(s_, c): WTt[c][:, s_].rearrange("p g hc i -> p (g hc i)")
                  for s_ in range(2) for c in range(2)}
            bs16 = cpool.tile([1, DG], GDT, tag="bs16")
            ones16 = cpool.tile([1, 128], GDT, tag="ones16")

            x_cm = x_d.ap().rearrange("(n p) (c q) -> p n c q", p=128, q=128)
            h_cm = h_d.ap().rearrange("(n p) (c q) -> p n c q", p=128, q=128)
            k_t = k_d.ap().rearrange("(n p) i -> p n i", p=128)
            kp_t = kp_d.ap().rearrange("(n p) i -> p n i", p=128)
            ho_t = ho_d.ap().rearrange("(n p) i -> p n i", p=128)
            ko_t = ko_d.ap().rearrange("(n p) i -> p n i", p=128)
            kpo_t = kpo_d.ap().rearrange("(n p) i -> p n i", p=128)

            with tc.tile_pool(name="io", bufs=2) as io, \
                 tc.tile_pool(name="work", bufs=2) as work, \
                 tc.tile_pool(name="tr", bufs=TR_BUFS) as tr, \
                 tc.tile_pool(name="psum", bufs=PSUM_BUFS, space="PSUM") as pp, \
                 tc.tile_pool(name="wload", bufs=1) as wload:
                # -- weights first, as f32 split across BOTH HWDGE rings
                #    (scalar: i-chunk 0, sync: i-chunk 1) so they drain in
                #    parallel with the SWDGE ring carrying group-0 x/h;
                #    DVE-cast to bf16 afterwards.
                stg32 = [wload.tile([128, 2, G5, 2, 128], F32,
                                    name=f"stg32_{c}", tag=f"stg32_{c}")
                         for c in range(2)]
                stc = [wload.tile([128, 2, G5, 2, 128], GDT, name=f"stc{c}",
                                  tag=f"stc{c}")
                       for c in range(2)]
                wsrc = [w.ap().rearrange("g (hc p) (c i) -> p c g hc i",
                                         p=128, i=128)
                        for w in (wx_d, uh_d)]
                for s_ in range(2):
                    nc.scalar.dma_start(stg32[0][:, s_], wsrc[s_][:, 0])
                    nc.sync.dma_start(stg32[1][:, s_], wsrc[s_][:, 1])
                xh16_0 = io.tile([128, 2, GROUP, 2, 128], GDT, tag="xh16")
                nc.gpsimd.dma_start(xh16_0[:, 0], x_cm[:, 0:GROUP])
                nc.gpsimd.dma_start(xh16_0[:, 1], h_cm[:, 0:GROUP])
                xhT_0 = tr.tile([128, 2, GROUP, 2, 128], GDT, tag="xhT")
                nc.sync.dma_start(xhT_0[:], xh16_0[:], transpose=True)
                # cast + tanh-gate (g=4) x2 scale (sigmoid folding)
                for c in range(2):
                    nc.vector.tensor_copy(stc[c][:], stg32[c][:])
                    nc.vector.tensor_scalar_mul(stc[c][:, :, 4],
                                                stc[c][:, :, 4], 2.0)
                    nc.sync.dma_start(WTt[c][:], stc[c][:], transpose=True)

                # -- bias row: bs16 = bx + bh (bf16), tanh gate x2
                bxr = wload.tile([1, DG], GDT, tag="bxr")
                bhr = wload.tile([1, DG], GDT, tag="bhr")
                nc.gpsimd.dma_start(
                    bxr[:], bx_d.ap().rearrange("g h -> (g h)").unsqueeze(0))
                nc.gpsimd.dma_start(
                    bhr[:], bh_d.ap().rearrange("g h -> (g h)").unsqueeze(0))
                nc.vector.tensor_add(bs16[:], bxr[:], bhr[:])
                nc.vector.tensor_scalar_mul(bs16[:, 4 * H:], bs16[:, 4 * H:],
                                            2.0)
                nc.vector.memset(ones16[:], 1.0)

                # -- main loop
                for gi in range(NG):
                    nsl = slice(gi * GROUP, (gi + 1) * GROUP)
                    if gi == 0:
                        xhT = xhT_0
                    else:
                        xh16 = io.tile([128, 2, GROUP, 2, 128], GDT,
                                       name=f"xh16_{gi}", tag="xh16")
                        nc.gpsimd.dma_start(xh16[:, 0], x_cm[:, nsl])
                        nc.gpsimd.dma_start(xh16[:, 1], h_cm[:, nsl])
                    kr = io.tile([128, GROUP, H], DT, tag="kr")
                    kpp = io.tile([128, GROUP, H], DT, tag="kpp")
                    nc.gpsimd.dma_start(kr[:], k_t[:, nsl, :])
                    nc.gpsimd.dma_start(kpp[:], kp_t[:, nsl, :])
                    kp_o = io.tile([128, GROUP, H], DT, tag="kp_o")
                    k_o = io.tile([128, GROUP, H], DT, tag="k_o")
                    h_o = io.tile([128, GROUP, H], DT, tag="h_o")

                    if gi > 0:
                        xhT = tr.tile([128, 2, GROUP, 2, 128], GDT,
                                      name=f"xhT_{gi}", tag="xhT")
                        nc.sync.dma_start(xhT[:], xh16[:], transpose=True)

                    gates = work.tile([128, GROUP, DG], DT, tag="gates")

                    for j in range(GROUP):
                        ps = pp.tile([128, DG], F32, tag="ps")
                        for n0 in range(0, DG, 512):
                            n1 = min(n0 + 512, DG)
                            nc.tensor.matmul(ps[:, n0:n1], ones16[:],
                                             bs16[:, n0:n1],
                                             start=True, stop=False)
                        for si in range(2):
                            for c in range(2):
                                lhsT = xhT[:, si, j, c]
                                last = si == 1 and c == 1
                                for n0 in range(0, DG, 512):
                                    n1 = min(n0 + 512, DG)
                                    nc.tensor.matmul(
                                        ps[:, n0:n1], lhsT,
                                        WT[si, c][:, n0:n1],
                                        start=False, stop=last)
                        # all 5 gates in one sigmoid (tanh gate pre-scaled)
                        nc.scalar.activation(gates[:, j, :], ps[:], AF.Sigmoid)

                    # ---- group elementwise tail, all fp16 ----
                    # Last group runs the tail in two halves so half A
                    # overlaps the final matmuls (shorter drain); other
                    # groups do one full-width pass (N=2048 DVE ops).
                    halves = ([(0, 4), (4, 8)] if gi == NG - 1
                              else [(0, GROUP)])
                    for lo, hi in halves:
                        hs = slice(lo, hi)
                        w_ = hi - lo
                        f_ = gates[:, hs, 0:256]
                        i_ = gates[:, hs, 256:512]
                        o_ = gates[:, hs, 512:768]
                        u_ = gates[:, hs, 768:1024]
                        s4 = gates[:, hs, 1024:1280]
                        c2 = work.tile([128, GROUP, H], DT, name="c2",
                                       tag="c2")
                        nc.vector.tensor_scalar(c2[:, :w_], s4, 2.0, -1.0,
                                                ALU.mult, ALU.add)
                        d = work.tile([128, GROUP, H], DT, name="d", tag="d")
                        nc.vector.tensor_sub(d[:, :w_], c2[:, :w_],
                                             kpp[:, hs])
                        e = work.tile([128, GROUP, H], DT, name="e", tag="e")
                        nc.vector.tensor_mul(e[:, :w_], u_, d[:, :w_])
                        nc.vector.tensor_add(kp_o[:, hs], e[:, :w_],
                                             kpp[:, hs])
                        m = work.tile([128, GROUP, H], DT, name="m", tag="d")
                        nc.vector.tensor_mul(m[:, :w_], f_, kr[:, hs])
                        n = work.tile([128, GROUP, H], DT, name="n", tag="e")
                        nc.vector.tensor_mul(n[:, :w_], i_, kp_o[:, hs])
                        nc.vector.tensor_add(k_o[:, hs], m[:, :w_],
                                             n[:, :w_])
                        tk = work.tile([128, GROUP, H], DT, name="tk",
                                       tag="c2")
                        nc.scalar.activation(tk[:, :w_], k_o[:, hs], AF.Tanh)
                        nc.vector.tensor_mul(h_o[:, hs], o_, tk[:, :w_])

                    nc.scalar.dma_start(kpo_t[:, nsl, :], kp_o[:])
                    nc.scalar.dma_start(ko_t[:, nsl, :], k_o[:])
                    nc.scalar.dma_start(ho_t[:, nsl, :], h_o[:])

    nc.compile()
    _CACHE["nc"] = nc
    return nc


def make_in_maps(np_inputs):
    x = np.asarray(np_inputs["x"], dtype=np.float32)
    h_prev = np.asarray(np_inputs["h_prev"], dtype=np.float32)
    k_prev = np.asarray(np_inputs["k_prev"], dtype=np.float32)
    kp_prev = np.asarray(np_inputs["kp_prev"], dtype=np.float32)
    Wx = np.ascontiguousarray(np.asarray(np_inputs["Wx"], dtype=np.float32))
    bx = np.ascontiguousarray(np.asarray(np_inputs["bx"], dtype=np.float32))
    Uh = np.ascontiguousarray(np.asarray(np_inputs["Uh"], dtype=np.float32))
    bh = np.ascontiguousarray(np.asarray(np_inputs["bh"], dtype=np.float32))
    in_maps = []
    for c in range(N_CORES):
        sl = slice(c * BL, (c + 1) * BL)
        in_maps.append({
            "x": np.ascontiguousarray(x[sl]),
            "h_prev": np.ascontiguousarray(h_prev[sl]),
            "k_prev": np.ascontiguousarray(k_prev[sl]),
            "kp_prev": np.ascontiguousarray(kp_prev[sl]),
            "Wx": Wx, "bx": bx, "Uh": Uh, "bh": bh,
        })
    return in_maps


def kernel(x, h_prev, k_prev, kp_prev, Wx, bx, Uh, bh):
    nc = _build()
    in_maps = make_in_maps(dict(x=x, h_prev=h_prev, k_prev=k_prev,
                                kp_prev=kp_prev, Wx=Wx, bx=bx, Uh=Uh, bh=bh))
    res = run_bass_kernel_spmd(nc, in_maps, list(range(N_CORES)))
    h_out = np.concatenate(
        [np.asarray(res.results[c]["h_out"]).astype(np.float32)
         for c in range(N_CORES)], axis=0)
    k_out = np.concatenate(
        [np.asarray(res.results[c]["k_out"]).astype(np.float32)
         for c in range(N_CORES)], axis=0)
    kp_out = np.concatenate(
        [np.asarray(res.results[c]["kp_out"]).astype(np.float32)
         for c in range(N_CORES)], axis=0)
    return (h_out, k_out, kp_out)
